# revision 2
# baseline (speedup 1.0000x reference)
"""Trainium2 Bass kernel for causal self-attention with segment masking.

Sharding: 8 cores = 2 batches x 4 head-groups (4 heads each).
Per core: QKV projection (bf16), S^T-layout attention with data-dependent
tile skipping AND per-tile q-column windowing (causal + segment structure),
output projection producing a partial [T, D] sum; host adds the 4 partials
per batch.

Layouts (per core):
  x_T   [D, T]      bf16  (host-transposed)
  q_T/k_T [128, T]  bf16  two tiles, one per head pair (2 heads x 64 dims)
  v_ext [128, 16kb, 4h, 65] bf16 (col 64 = ones -> softmax denominator)
  s     [128k, 2h, 512q] f32 PSUM, written only on the tile's live window
  pt    [128, 2, 512] bf16 SBUF = exp(s/8) * mask01 (window only)
  y_ps  [65, 512]   f32 PSUM = v_ext.T @ pt (row 64 = sum of p = denom);
        ragged window accumulation relies on per-element has_written bits
  y_qc  [128, 2, T] bf16 (normalized, feeds proj as lhsT)
"""

import numpy as np
import ml_dtypes

import concourse.bass as bass
import concourse.mybir as mybir
import concourse.tile as tile
from concourse import bacc
from concourse import bass_utils

B, T, D = 2, 2048, 1024
H, HD = 16, 64
QC = 512            # q chunk (max matmul free dim)
KB = 128            # k block (partition dim)
NQC = T // QC       # 4
NKB = T // KB       # 16
DK = D // 128       # 8 contraction chunks for projections
BF16 = mybir.dt.bfloat16
F32 = mybir.dt.float32
nbf = ml_dtypes.bfloat16
Exp = mybir.ActivationFunctionType.Exp


def _schedule(seg):
    """Data-dependent tile schedule, shared (union) across both batches.

    Returns (act, mask_arrs, wtot):
      act: {qc: [(kb, w0, w1, moff)]} where [w0, w1) is the live q-column
           window within the chunk and moff the column offset of this
           tile's mask in the packed mask tensor (-1 = all-ones window).
      mask_arrs: per-batch packed bf16 {0,1} masks [KB, wtot].
    """
    ar = np.arange(T)
    masks = [
        (seg[b][:, None] == seg[b][None, :]) & (ar[:, None] <= ar[None, :])
        for b in range(B)
    ]  # mask_T[k, q]
    act = {qc: [] for qc in range(NQC)}
    mask_cols = [[] for _ in range(B)]
    wtot = 0
    for qc in range(NQC):
        for kb in range(NKB):
            if kb * KB > qc * QC + QC - 1:
                continue  # fully above the diagonal
            subs = [
                masks[b][kb * KB:(kb + 1) * KB, qc * QC:(qc + 1) * QC]
                for b in range(B)
            ]
            u = subs[0] | subs[1]
            if not u.any():
                continue  # dead tile in both batches: skip entirely
            idx = np.nonzero(u.any(axis=0))[0]
            w0 = int(idx[0]) & ~3
            w1 = min(QC, (int(idx[-1]) + 4) & ~3)
            win = [s[:, w0:w1] for s in subs]
            if all(w.all() for w in win):
                act[qc].append((kb, w0, w1, -1))
            else:
                act[qc].append((kb, w0, w1, wtot))
                for b in range(B):
                    mask_cols[b].append(win[b].astype(nbf))
                wtot += w1 - w0
    if wtot == 0:
        wtot = 4
        mask_arrs = [np.zeros((KB, 4), nbf) for _ in range(B)]
    else:
        mask_arrs = [
            np.ascontiguousarray(np.concatenate(mask_cols[b], axis=1))
            for b in range(B)
        ]
    return act, mask_arrs, wtot


def _build(act, wtot):
    nc = bacc.Bacc("TRN2", target_bir_lowering=False, debug=False, num_devices=8)
    xT = nc.dram_tensor("xT", [D, T], BF16, kind="ExternalInput").ap()
    wqkv = nc.dram_tensor("wqkv", [D, 768], BF16, kind="ExternalInput").ap()
    wp = nc.dram_tensor("wp", [256, D], BF16, kind="ExternalInput").ap()
    mk = nc.dram_tensor("mask", [KB, wtot], BF16, kind="ExternalInput").ap()
    out = nc.dram_tensor("out", [T, D], BF16, kind="ExternalOutput").ap()

    with tile.TileContext(nc) as tc:
        with (
            tc.tile_pool(name="const", bufs=1) as cpool,
            tc.tile_pool(name="ptp", bufs=3) as ppool,
            tc.tile_pool(name="otp", bufs=3) as opool,
            tc.tile_pool(name="nrm", bufs=2) as npool,
            tc.tile_pool(name="psq", bufs=2, space="PSUM") as psq,
            tc.tile_pool(name="pss", bufs=2, space="PSUM") as pss,
            tc.tile_pool(name="psy", bufs=2, space="PSUM") as psy,
        ):
            # ---- input DMAs ----
            # sync queue: x qc0, mask, then half of x qc1-3
            # scalar queue: wqkv, wp, other half of x qc1-3
            wqkv_sb = cpool.tile([128, DK, 768], BF16, tag="wqkv")
            x_sb = cpool.tile([128, DK, T], BF16, tag="x")
            for i in range(DK):
                nc.sync.dma_start(
                    x_sb[:, i, 0:QC], xT[i * 128:(i + 1) * 128, 0:QC])
                nc.scalar.dma_start(
                    wqkv_sb[:, i, :], wqkv[i * 128:(i + 1) * 128, :])
            mask_sb = cpool.tile([128, wtot], BF16, tag="m")
            nc.sync.dma_start(mask_sb[:], mk)
            wp_sb = cpool.tile([128, 2, D], BF16, tag="wp")
            nc.scalar.dma_start(wp_sb[:], wp.rearrange("(c p) n -> p c n", p=128))
            for qc in range(1, NQC):
                for i in range(DK):
                    eng = nc.sync if i % 2 == 0 else nc.scalar
                    eng.dma_start(
                        x_sb[:, i, qc * QC:(qc + 1) * QC],
                        xT[i * 128:(i + 1) * 128, qc * QC:(qc + 1) * QC],
                    )

            q_sb = [cpool.tile([128, T], BF16, tag=f"q{p}", name=f"q{p}") for p in range(2)]
            k_sb = [cpool.tile([128, T], BF16, tag=f"k{p}", name=f"k{p}") for p in range(2)]
            v_sb = cpool.tile([128, NKB, 4, 65], BF16, tag="v")
            y_qc = [cpool.tile([128, 2, QC], BF16, tag=f"y{qc}", name=f"y{qc}") for qc in range(NQC)]
            nc.vector.memset(v_sb[:, :, :, 64], 1.0)

            # PE warm-up burn: junk matmuls on the first weight chunk while
            # x DMAs land, so the HAM clock-gate opens before real work.
            warm = psq.tile([128, 512], F32, tag="psq", name="warm")
            for _ in range(10):
                nc.tensor.matmul(
                    warm[:], wqkv_sb[:, 0, 0:128], wqkv_sb[:, 0, 0:512],
                    start=True, stop=True,
                )

            # ---- building blocks ----
            def emit_qkv_qk(qc):
                for p in range(2):
                    ps = psq.tile([128, 512], F32, tag="psq", name=f"q_{qc}_{p}")
                    for i in range(DK):
                        nc.tensor.matmul(
                            ps[:], wqkv_sb[:, i, p * 128:(p + 1) * 128],
                            x_sb[:, i, qc * 512:(qc + 1) * 512],
                            start=(i == 0), stop=(i == DK - 1),
                        )
                    nc.vector.tensor_copy(out=q_sb[p][:, qc * 512:(qc + 1) * 512], in_=ps[:])
                for p in range(2):
                    ps = psq.tile([128, 512], F32, tag="psq", name=f"k_{qc}_{p}")
                    for i in range(DK):
                        nc.tensor.matmul(
                            ps[:], wqkv_sb[:, i, 256 + p * 128:256 + (p + 1) * 128],
                            x_sb[:, i, qc * 512:(qc + 1) * 512],
                            start=(i == 0), stop=(i == DK - 1),
                        )
                    nc.scalar.copy(out=k_sb[p][:, qc * 512:(qc + 1) * 512], in_=ps[:])

            def emit_qkv_v(qc):
                for kb in range(qc * 4, qc * 4 + 4):
                    ps = psq.tile([128, 512], F32, tag="psq", name=f"v_{kb}")
                    for i in range(DK):
                        nc.tensor.matmul(
                            ps[:, 0:256], x_sb[:, i, kb * 128:(kb + 1) * 128],
                            wqkv_sb[:, i, 512:768],
                            start=(i == 0), stop=(i == DK - 1),
                        )
                    nc.vector.tensor_copy(
                        out=v_sb[:, kb, :, 0:64],
                        in_=ps[:, 0:256].rearrange("p (h d) -> p h d", h=4),
                    )

            def emit_attn(qc, p):
                kbs = act[qc]
                y_ps = [psy.tile([65, 512], F32, tag="psy", name=f"yps{p}_{qc}_{hh}") for hh in range(2)]
                for idx, (kb, w0, w1, moff) in enumerate(kbs):
                    w = w1 - w0
                    first, last = idx == 0, idx == len(kbs) - 1
                    s_ps = pss.tile([128, 2, 512], F32, tag="pss", name=f"s_{p}_{qc}_{kb}")
                    for hh in range(2):
                        lo = hh * 64
                        nc.tensor.matmul(
                            s_ps[:, hh, w0:w1],
                            k_sb[p][lo:lo + 64, kb * 128:(kb + 1) * 128],
                            q_sb[p][lo:lo + 64, qc * 512 + w0:qc * 512 + w1],
                            start=True, stop=True,
                        )
                    pt = ppool.tile([128, 2, 512], BF16, tag="pt", name=f"pt{p}_{qc}_{kb}")
                    nc.scalar.activation(pt[:, :, w0:w1], s_ps[:, :, w0:w1], Exp, scale=0.125)
                    if moff >= 0:
                        nc.gpsimd.tensor_tensor(
                            out=pt[:, :, w0:w1],
                            in0=pt[:, :, w0:w1],
                            in1=mask_sb[:, None, moff:moff + w].to_broadcast((128, 2, w)),
                            op=mybir.AluOpType.mult,
                        )
                    for hh in range(2):
                        nc.tensor.matmul(
                            y_ps[hh][:, w0:w1], v_sb[:, kb, p * 2 + hh, :],
                            pt[:, hh, w0:w1],
                            start=first, stop=last, skip_group_check=True,
                        )
                # normalization: denom rows -> [128,8] -> recip -> broadcast
                lr = npool.tile([1, 2, 512], F32, tag="lr", name=f"lr{qc}_{p}")
                nc.vector.tensor_copy(out=lr[:, 0, :], in_=y_ps[0][64:65, :])
                nc.scalar.copy(out=lr[:, 1, :], in_=y_ps[1][64:65, :])
                lp = npool.tile([128, 8], F32, tag="lp")
                nc.sync.dma_start(lp[:], lr[:])
                nc.vector.reciprocal(lp[:], lp[:])
                l0 = npool.tile([1, 1024], F32, tag="l0")
                nc.sync.dma_start(l0[:], lp[:])
                lb = npool.tile([64, 1024], F32, tag="lb")
                nc.gpsimd.partition_broadcast(lb[:], l0[:])
                nc.vector.tensor_mul(
                    out=y_qc[qc][0:64, p, :], in0=y_ps[0][0:64, :], in1=lb[:, 0:512])
                yt = npool.tile([64, 512], BF16, tag="yt")
                nc.vector.tensor_mul(out=yt[:], in0=y_ps[1][0:64, :], in1=lb[:, 512:1024])
                nc.sync.dma_start(y_qc[qc][64:128, p, :], yt[:])

            def emit_proj(qc):
                for mt in range(qc * 4, qc * 4 + 4):
                    ot = opool.tile([128, 1024], BF16, tag="ot", name=f"ot{mt}")
                    for n in range(2):
                        ps = psq.tile([128, 512], F32, tag="psq", name=f"pso{mt}_{n}")
                        for c in range(2):
                            nc.tensor.matmul(
                                ps[:], y_qc[qc][:, c, (mt % 4) * 128:(mt % 4) * 128 + 128],
                                wp_sb[:, c, n * 512:(n + 1) * 512],
                                start=(c == 0), stop=(c == 1),
                            )
                        if n == 0:
                            nc.vector.tensor_copy(out=ot[:, 0:512], in_=ps[:])
                        else:
                            nc.scalar.copy(out=ot[:, 512:1024], in_=ps[:])
                    nc.gpsimd.dma_start(out[mt * 128:(mt + 1) * 128, :], ot[:])

            # ---- schedule ----
            emit_qkv_qk(0)
            emit_qkv_v(0)
            emit_attn(0, 0)
            emit_qkv_qk(1)
            emit_attn(0, 1)
            emit_qkv_v(1)
            emit_attn(1, 0)
            emit_qkv_qk(2)
            emit_attn(1, 1)
            emit_qkv_v(2)
            emit_proj(0)
            emit_attn(2, 0)
            emit_qkv_qk(3)
            emit_attn(2, 1)
            emit_qkv_v(3)
            emit_proj(1)
            emit_attn(3, 0)
            emit_proj(2)
            emit_attn(3, 1)
            emit_proj(3)

    nc.compile()
    return nc


def _in_maps(x, seg, Wqkv, Wproj, mask_arrs):
    maps = []
    for c in range(8):
        b, g = divmod(c, 4)
        h0 = g * 4
        cs, ce = h0 * 64, h0 * 64 + 256
        maps.append({
            "xT": np.ascontiguousarray(x[b].T).astype(nbf),
            "wqkv": np.ascontiguousarray(np.concatenate(
                [Wqkv[:, cs:ce], Wqkv[:, D + cs:D + ce], Wqkv[:, 2 * D + cs:2 * D + ce]],
                axis=1)).astype(nbf),
            "wp": np.ascontiguousarray(Wproj[cs:ce, :]).astype(nbf),
            "mask": mask_arrs[b],
        })
    return maps


_CACHE = {}


def _prepare(x, segment_ids, W_qkv, W_proj):
    x = np.asarray(x, np.float32)
    seg = np.asarray(segment_ids)
    Wqkv = np.asarray(W_qkv, np.float32)
    Wproj = np.asarray(W_proj, np.float32)
    tiles, mask_arrs, wtot = _schedule(seg)
    key = (tuple((qc, t) for qc in tiles for t in tiles[qc]), wtot)
    if key not in _CACHE:
        _CACHE[key] = _build(tiles, wtot)
    nc = _CACHE[key]
    return nc, _in_maps(x, seg, Wqkv, Wproj, mask_arrs)


def kernel(x, segment_ids, W_qkv, W_proj):
    nc, in_maps = _prepare(x, segment_ids, W_qkv, W_proj)
    res = bass_utils.run_bass_kernel_spmd(nc, in_maps, core_ids=list(range(8)))
    out = np.zeros((B, T, D), np.float32)
    for c in range(8):
        out[c // 4] += res.results[c]["out"].astype(np.float32)
    return out


# revision 6
# speedup vs baseline: 1.3400x; 1.3400x over previous
"""Trainium2 Bass kernel for causal self-attention with segment masking.

Sharding: 8 cores = 2 batches x 4 head-groups (4 heads each).
Per core: QKV projection (bf16), S^T-layout attention with data-dependent
tile skipping AND per-tile q-column windowing (causal + segment structure),
output projection producing a partial [T, D] sum; host adds the 4 partials
per batch.

Layouts (per core):
  x_T   [D, T]      bf16  (host-transposed)
  q_T/k_T [128, T]  bf16  two tiles, one per head pair (2 heads x 64 dims)
  v_ext [128, 16kb, 4h, 65] bf16 (col 64 = ones -> softmax denominator)
  s     [128k, 2h, 512q] f32 PSUM, written only on the tile's live window
  pt    [128, 2, 512] bf16 SBUF = exp(s/8) * mask01 (window only)
  y_ps  [65, 512]   f32 PSUM = v_ext.T @ pt (row 64 = sum of p = denom);
        ragged window accumulation relies on per-element has_written bits
  y_qc  [128, 2, T] bf16 (normalized, feeds proj as lhsT)
"""

import numpy as np
import ml_dtypes

import concourse.bass as bass
import concourse.mybir as mybir
import concourse.tile as tile
from concourse import bacc
from concourse import bass_utils

B, T, D = 2, 2048, 1024
H, HD = 16, 64
QC = 512            # q chunk (max matmul free dim)
KB = 128            # k block (partition dim)
NQC = T // QC       # 4
NKB = T // KB       # 16
DK = D // 128       # 8 contraction chunks for projections
BF16 = mybir.dt.bfloat16
F32 = mybir.dt.float32
nbf = ml_dtypes.bfloat16
Exp = mybir.ActivationFunctionType.Exp


def _schedule(seg):
    """Data-dependent tile schedule, shared (union) across both batches.

    Returns (act, mask_arrs, wtot):
      act: {qc: [(kb, w0, w1, moff)]} where [w0, w1) is the live q-column
           window within the chunk and moff the column offset of this
           tile's mask in the packed mask tensor (-1 = all-ones window).
      mask_arrs: per-batch packed bf16 {0,1} masks [KB, wtot].
    """
    ar = np.arange(T)
    masks = [
        (seg[b][:, None] == seg[b][None, :]) & (ar[:, None] <= ar[None, :])
        for b in range(B)
    ]  # mask_T[k, q]
    act = {qc: [] for qc in range(NQC)}
    mask_cols = [[] for _ in range(B)]
    wtot = 0
    for qc in range(NQC):
        for kb in range(NKB):
            if kb * KB > qc * QC + QC - 1:
                continue  # fully above the diagonal
            subs = [
                masks[b][kb * KB:(kb + 1) * KB, qc * QC:(qc + 1) * QC]
                for b in range(B)
            ]
            u = subs[0] | subs[1]
            if not u.any():
                continue  # dead tile in both batches: skip entirely
            idx = np.nonzero(u.any(axis=0))[0]
            w0 = int(idx[0]) & ~3
            w1 = min(QC, (int(idx[-1]) + 4) & ~3)
            win = [s[:, w0:w1] for s in subs]
            if all(w.all() for w in win):
                act[qc].append((kb, w0, w1, -1))
            else:
                act[qc].append((kb, w0, w1, wtot))
                for b in range(B):
                    mask_cols[b].append(win[b].astype(nbf))
                wtot += w1 - w0
    if wtot == 0:
        wtot = 4
        mask_arrs = [np.zeros((KB, 4), nbf) for _ in range(B)]
    else:
        mask_arrs = [
            np.ascontiguousarray(np.concatenate(mask_cols[b], axis=1))
            for b in range(B)
        ]
    return act, mask_arrs, wtot


def _build(act, wtot):
    nc = bacc.Bacc("TRN2", target_bir_lowering=False, debug=False, num_devices=8)
    xT = nc.dram_tensor("xT", [D, T], BF16, kind="ExternalInput").ap()
    wqkv = nc.dram_tensor("wqkv", [D, 768], BF16, kind="ExternalInput").ap()
    wp = nc.dram_tensor("wp", [256, D], BF16, kind="ExternalInput").ap()
    mk = nc.dram_tensor("mask", [KB, wtot], BF16, kind="ExternalInput").ap()
    out = nc.dram_tensor("out", [T, D], BF16, kind="ExternalOutput").ap()

    with tile.TileContext(nc) as tc:
        with (
            tc.tile_pool(name="const", bufs=1) as cpool,
            tc.tile_pool(name="ptp", bufs=3) as ppool,
            tc.tile_pool(name="otp", bufs=3) as opool,
            tc.tile_pool(name="nrm", bufs=2) as npool,
            tc.tile_pool(name="psq", bufs=2, space="PSUM") as psq,
            tc.tile_pool(name="pss", bufs=2, space="PSUM") as pss,
            tc.tile_pool(name="psy", bufs=2, space="PSUM") as psy,
        ):
            # ---- input DMAs ----
            # sync queue: x qc0, mask, then half of x qc1-3
            # scalar queue: wqkv, wp, other half of x qc1-3
            wqkv_sb = cpool.tile([128, DK, 768], BF16, tag="wqkv")
            x_sb = cpool.tile([128, DK, T], BF16, tag="x")
            for i in range(DK):
                nc.sync.dma_start(
                    x_sb[:, i, 0:QC], xT[i * 128:(i + 1) * 128, 0:QC])
                nc.scalar.dma_start(
                    wqkv_sb[:, i, :], wqkv[i * 128:(i + 1) * 128, :])
            mask_sb = cpool.tile([128, wtot], BF16, tag="m")
            nc.sync.dma_start(mask_sb[:], mk)
            wp_sb = cpool.tile([128, 2, D], BF16, tag="wp")
            nc.scalar.dma_start(wp_sb[:], wp.rearrange("(c p) n -> p c n", p=128))
            for qc in range(1, NQC):
                for i in range(DK):
                    eng = nc.sync if i % 2 == 0 else nc.scalar
                    eng.dma_start(
                        x_sb[:, i, qc * QC:(qc + 1) * QC],
                        xT[i * 128:(i + 1) * 128, qc * QC:(qc + 1) * QC],
                    )

            q_sb = [cpool.tile([128, T], BF16, tag=f"q{p}", name=f"q{p}") for p in range(2)]
            k_sb = [cpool.tile([128, T], BF16, tag=f"k{p}", name=f"k{p}") for p in range(2)]
            v_sb = cpool.tile([128, NKB, 4, 65], BF16, tag="v")
            y_qc = [cpool.tile([128, 2, QC], BF16, tag=f"y{qc}", name=f"y{qc}") for qc in range(NQC)]
            nc.vector.memset(v_sb[:, :, :, 64], 1.0)

            # PE warm-up burn: junk matmuls on the first weight chunk while
            # x DMAs land, so the HAM clock-gate opens before real work.
            warm = psq.tile([128, 512], F32, tag="psq", name="warm")
            for _ in range(10):
                nc.tensor.matmul(
                    warm[:], wqkv_sb[:, 0, 0:128], wqkv_sb[:, 0, 0:512],
                    start=True, stop=True,
                )

            # ---- building blocks ----
            def emit_qkv_qk(qc):
                for p in range(2):
                    ps = psq.tile([128, 512], F32, tag="psq", name=f"q_{qc}_{p}")
                    for i in range(DK):
                        nc.tensor.matmul(
                            ps[:], wqkv_sb[:, i, p * 128:(p + 1) * 128],
                            x_sb[:, i, qc * 512:(qc + 1) * 512],
                            start=(i == 0), stop=(i == DK - 1),
                        )
                    nc.vector.tensor_copy(out=q_sb[p][:, qc * 512:(qc + 1) * 512], in_=ps[:])
                for p in range(2):
                    ps = psq.tile([128, 512], F32, tag="psq", name=f"k_{qc}_{p}")
                    for i in range(DK):
                        nc.tensor.matmul(
                            ps[:], wqkv_sb[:, i, 256 + p * 128:256 + (p + 1) * 128],
                            x_sb[:, i, qc * 512:(qc + 1) * 512],
                            start=(i == 0), stop=(i == DK - 1),
                        )
                    nc.vector.tensor_copy(out=k_sb[p][:, qc * 512:(qc + 1) * 512], in_=ps[:])

            def emit_qkv_v(qc):
                for kb in range(qc * 4, qc * 4 + 4):
                    ps = psq.tile([128, 512], F32, tag="psq", name=f"v_{kb}")
                    for i in range(DK):
                        nc.tensor.matmul(
                            ps[:, 0:256], x_sb[:, i, kb * 128:(kb + 1) * 128],
                            wqkv_sb[:, i, 512:768],
                            start=(i == 0), stop=(i == DK - 1),
                        )
                    nc.vector.tensor_copy(
                        out=v_sb[:, kb, :, 0:64],
                        in_=ps[:, 0:256].rearrange("p (h d) -> p h d", h=4),
                    )

            def emit_attn(qc, p):
                kbs = act[qc]
                y_ps = [psy.tile([65, 512], F32, tag="psy", name=f"yps{p}_{qc}_{hh}") for hh in range(2)]
                for idx, (kb, w0, w1, moff) in enumerate(kbs):
                    w = w1 - w0
                    first, last = idx == 0, idx == len(kbs) - 1
                    s_ps = pss.tile([128, 2, 512], F32, tag="pss", name=f"s_{p}_{qc}_{kb}")
                    for hh in range(2):
                        lo = hh * 64
                        nc.tensor.matmul(
                            s_ps[:, hh, w0:w1],
                            k_sb[p][lo:lo + 64, kb * 128:(kb + 1) * 128],
                            q_sb[p][lo:lo + 64, qc * 512 + w0:qc * 512 + w1],
                            start=True, stop=True,
                        )
                    pt = ppool.tile([128, 2, 512], BF16, tag="pt", name=f"pt{p}_{qc}_{kb}")
                    nc.scalar.activation(pt[:, :, w0:w1], s_ps[:, :, w0:w1], Exp, scale=0.125)
                    if moff >= 0:
                        nc.vector.tensor_tensor(
                            out=pt[:, :, w0:w1],
                            in0=pt[:, :, w0:w1],
                            in1=mask_sb[:, None, moff:moff + w].to_broadcast((128, 2, w)),
                            op=mybir.AluOpType.mult,
                        )
                    for hh in range(2):
                        nc.tensor.matmul(
                            y_ps[hh][:, w0:w1], v_sb[:, kb, p * 2 + hh, :],
                            pt[:, hh, w0:w1],
                            start=first, stop=last, skip_group_check=True,
                        )
                # evacuate y to SBUF fast (releases the PSUM banks), then
                # normalize from SBUF: row 64 holds the softmax denominators
                y_sb = npool.tile([65, 2, 512], F32, tag="ysb", name=f"ysb{qc}_{p}")
                nc.scalar.copy(out=y_sb[:, 0, :], in_=y_ps[0][:])
                nc.scalar.copy(out=y_sb[:, 1, :], in_=y_ps[1][:])
                lp = npool.tile([128, 8], F32, tag="lp")
                nc.sync.dma_start(lp[:], y_sb[64:65, :, :])
                nc.vector.reciprocal(lp[:], lp[:])
                l0 = npool.tile([1, 1024], F32, tag="l0")
                nc.sync.dma_start(l0[:], lp[:])
                lb = npool.tile([64, 1024], F32, tag="lb")
                nc.gpsimd.partition_broadcast(lb[:], l0[:])
                nc.gpsimd.tensor_tensor(
                    out=y_qc[qc][0:64, p, :], in0=y_sb[0:64, 0, :],
                    in1=lb[:, 0:512], op=mybir.AluOpType.mult)
                yt = npool.tile([64, 512], BF16, tag="yt")
                nc.gpsimd.tensor_tensor(
                    out=yt[:], in0=y_sb[0:64, 1, :],
                    in1=lb[:, 512:1024], op=mybir.AluOpType.mult)
                nc.sync.dma_start(y_qc[qc][64:128, p, :], yt[:])

            def emit_proj(qc):
                for mt in range(qc * 4, qc * 4 + 4):
                    ot = opool.tile([128, 1024], BF16, tag="ot", name=f"ot{mt}")
                    for n in range(2):
                        ps = psq.tile([128, 512], F32, tag="psq", name=f"pso{mt}_{n}")
                        for c in range(2):
                            nc.tensor.matmul(
                                ps[:], y_qc[qc][:, c, (mt % 4) * 128:(mt % 4) * 128 + 128],
                                wp_sb[:, c, n * 512:(n + 1) * 512],
                                start=(c == 0), stop=(c == 1),
                            )
                        if n == 0:
                            nc.vector.tensor_copy(out=ot[:, 0:512], in_=ps[:])
                        elif mt % 2 == 0:
                            nc.scalar.copy(out=ot[:, 512:1024], in_=ps[:])
                        else:
                            nc.vector.tensor_copy(out=ot[:, 512:1024], in_=ps[:])
                    eng = nc.sync if mt % 2 == 0 else nc.scalar
                    eng.dma_start(out[mt * 128:(mt + 1) * 128, :], ot[:])

            # ---- schedule ----
            emit_qkv_qk(0)
            emit_qkv_v(0)
            emit_attn(0, 0)
            emit_qkv_qk(1)
            emit_attn(0, 1)
            emit_qkv_v(1)
            emit_attn(1, 0)
            emit_qkv_qk(2)
            emit_attn(1, 1)
            emit_qkv_v(2)
            emit_proj(0)
            emit_attn(2, 0)
            emit_qkv_qk(3)
            emit_attn(2, 1)
            emit_qkv_v(3)
            emit_proj(1)
            emit_attn(3, 0)
            emit_proj(2)
            emit_attn(3, 1)
            emit_proj(3)

    nc.compile()
    return nc


def _in_maps(x, seg, Wqkv, Wproj, mask_arrs):
    maps = []
    for c in range(8):
        b, g = divmod(c, 4)
        h0 = g * 4
        cs, ce = h0 * 64, h0 * 64 + 256
        maps.append({
            "xT": np.ascontiguousarray(x[b].T).astype(nbf),
            "wqkv": np.ascontiguousarray(np.concatenate(
                [Wqkv[:, cs:ce], Wqkv[:, D + cs:D + ce], Wqkv[:, 2 * D + cs:2 * D + ce]],
                axis=1)).astype(nbf),
            "wp": np.ascontiguousarray(Wproj[cs:ce, :]).astype(nbf),
            "mask": mask_arrs[b],
        })
    return maps


_CACHE = {}


def _prepare(x, segment_ids, W_qkv, W_proj):
    x = np.asarray(x, np.float32)
    seg = np.asarray(segment_ids)
    Wqkv = np.asarray(W_qkv, np.float32)
    Wproj = np.asarray(W_proj, np.float32)
    tiles, mask_arrs, wtot = _schedule(seg)
    key = (tuple((qc, t) for qc in tiles for t in tiles[qc]), wtot)
    if key not in _CACHE:
        _CACHE[key] = _build(tiles, wtot)
    nc = _CACHE[key]
    return nc, _in_maps(x, seg, Wqkv, Wproj, mask_arrs)


def kernel(x, segment_ids, W_qkv, W_proj):
    nc, in_maps = _prepare(x, segment_ids, W_qkv, W_proj)
    res = bass_utils.run_bass_kernel_spmd(nc, in_maps, core_ids=list(range(8)))
    out = np.zeros((B, T, D), np.float32)
    for c in range(8):
        out[c // 4] += res.results[c]["out"].astype(np.float32)
    return out


# revision 11
# speedup vs baseline: 1.3533x; 1.0099x over previous
"""Trainium2 Bass kernel for causal self-attention with segment masking.

Sharding: 8 cores = 2 batches x 4 head-groups (4 heads each).
Per core: QKV projection (bf16), S^T-layout attention with data-dependent
tile skipping AND per-tile q-column windowing (causal + segment structure),
output projection producing a partial [T, D] sum; host adds the 4 partials
per batch.

Layouts (per core):
  x_T   [D, T]      bf16  (host-transposed)
  q_T/k_T [128, T]  bf16  two tiles, one per head pair (2 heads x 64 dims)
  v_ext [128, 16kb, 4h, 65] bf16 (col 64 = ones -> softmax denominator)
  s     [128k, 2h, 512q] f32 PSUM, written only on the tile's live window
  pt    [128, 2, 512] bf16 SBUF = exp(s/8) * mask01 (window only)
  y_ps  [65, 512]   f32 PSUM = v_ext.T @ pt (row 64 = sum of p = denom);
        ragged window accumulation relies on per-element has_written bits
  y_qc  [128, 2, T] bf16 (normalized, feeds proj as lhsT)
"""

import numpy as np
import ml_dtypes

import concourse.bass as bass
import concourse.mybir as mybir
import concourse.tile as tile
from concourse import bacc
from concourse import bass_utils

B, T, D = 2, 2048, 1024
H, HD = 16, 64
QC = 512            # q chunk (max matmul free dim)
KB = 128            # k block (partition dim)
NQC = T // QC       # 4
NKB = T // KB       # 16
DK = D // 128       # 8 contraction chunks for projections
BF16 = mybir.dt.bfloat16
F32 = mybir.dt.float32
nbf = ml_dtypes.bfloat16
Exp = mybir.ActivationFunctionType.Exp


def _schedule(seg):
    """Data-dependent tile schedule, shared (union) across both batches.

    Returns (act, mask_arrs, wtot):
      act: {qc: [(kb, w0, w1, moff)]} where [w0, w1) is the live q-column
           window within the chunk and moff the column offset of this
           tile's mask in the packed mask tensor (-1 = all-ones window).
      mask_arrs: per-batch packed bf16 {0,1} masks [KB, wtot].
    """
    ar = np.arange(T)
    masks = [
        (seg[b][:, None] == seg[b][None, :]) & (ar[:, None] <= ar[None, :])
        for b in range(B)
    ]  # mask_T[k, q]
    act = {qc: [] for qc in range(NQC)}
    mask_cols = [[] for _ in range(B)]
    wtot = 0
    for qc in range(NQC):
        for kb in range(NKB):
            if kb * KB > qc * QC + QC - 1:
                continue  # fully above the diagonal
            subs = [
                masks[b][kb * KB:(kb + 1) * KB, qc * QC:(qc + 1) * QC]
                for b in range(B)
            ]
            u = subs[0] | subs[1]
            if not u.any():
                continue  # dead tile in both batches: skip entirely
            idx = np.nonzero(u.any(axis=0))[0]
            w0 = int(idx[0]) & ~3
            w1 = min(QC, (int(idx[-1]) + 4) & ~3)
            win = [s[:, w0:w1] for s in subs]
            if all(w.all() for w in win):
                act[qc].append((kb, w0, w1, -1))
            else:
                act[qc].append((kb, w0, w1, wtot))
                for b in range(B):
                    mask_cols[b].append(win[b].astype(nbf))
                wtot += w1 - w0
    if wtot == 0:
        wtot = 4
        mask_arrs = [np.zeros((KB, 4), nbf) for _ in range(B)]
    else:
        mask_arrs = [
            np.ascontiguousarray(np.concatenate(mask_cols[b], axis=1))
            for b in range(B)
        ]
    return act, mask_arrs, wtot


def _build(act, wtot):
    nc = bacc.Bacc("TRN2", target_bir_lowering=False, debug=False, num_devices=8)
    xT = nc.dram_tensor("xT", [D, T], BF16, kind="ExternalInput").ap()
    wqkv = nc.dram_tensor("wqkv", [D, 768], BF16, kind="ExternalInput").ap()
    wp = nc.dram_tensor("wp", [256, D], BF16, kind="ExternalInput").ap()
    mk = nc.dram_tensor("mask", [KB, wtot], BF16, kind="ExternalInput").ap()
    out = nc.dram_tensor("out", [T, D], BF16, kind="ExternalOutput").ap()

    with tile.TileContext(nc) as tc:
        with (
            tc.tile_pool(name="const", bufs=1) as cpool,
            tc.tile_pool(name="ptp", bufs=3) as ppool,
            tc.tile_pool(name="otp", bufs=3) as opool,
            tc.tile_pool(name="nrm", bufs=2) as npool,
            tc.tile_pool(name="psq", bufs=2, space="PSUM") as psq,
            tc.tile_pool(name="pss", bufs=2, space="PSUM") as pss,
            tc.tile_pool(name="psy", bufs=2, space="PSUM") as psy,
        ):
            # ---- input DMAs (all on the sync HWDGE queue, priority order:
            # wqkv+x[qc0] interleaved, mask, wp, then x[qc1..3]) ----
            wqkv_sb = cpool.tile([128, DK, 768], BF16, tag="wqkv")
            x_sb = cpool.tile([128, DK, T], BF16, tag="x")
            for i in range(DK):
                nc.sync.dma_start(
                    wqkv_sb[:, i, :], wqkv[i * 128:(i + 1) * 128, :])
                nc.sync.dma_start(
                    x_sb[:, i, 0:QC], xT[i * 128:(i + 1) * 128, 0:QC])
            mask_sb = cpool.tile([128, wtot], BF16, tag="m")
            nc.sync.dma_start(mask_sb[:], mk)
            wp_sb = cpool.tile([128, 2, D], BF16, tag="wp")
            nc.sync.dma_start(wp_sb[:], wp.rearrange("(c p) n -> p c n", p=128))
            for qc in range(1, NQC):
                for i in range(DK):
                    nc.sync.dma_start(
                        x_sb[:, i, qc * QC:(qc + 1) * QC],
                        xT[i * 128:(i + 1) * 128, qc * QC:(qc + 1) * QC],
                    )

            q_sb = [cpool.tile([128, T], BF16, tag=f"q{p}", name=f"q{p}") for p in range(2)]
            k_sb = [cpool.tile([128, T], BF16, tag=f"k{p}", name=f"k{p}") for p in range(2)]
            v_sb = cpool.tile([128, NKB, 4, 65], BF16, tag="v")
            y_qc = [cpool.tile([128, 2, QC], BF16, tag=f"y{qc}", name=f"y{qc}") for qc in range(NQC)]
            nc.vector.memset(v_sb[:, :, :, 64], 1.0)

            # PE warm-up burn: junk matmuls on the first weight chunk while
            # x DMAs land, so the HAM clock-gate opens before real work.
            warm = psq.tile([128, 512], F32, tag="psq", name="warm")
            for _ in range(24):
                nc.tensor.matmul(
                    warm[:], wqkv_sb[:, 0, 0:128], wqkv_sb[:, 0, 0:512],
                    start=True, stop=True,
                )

            # ---- building blocks ----
            def emit_qkv_qk(qc):
                for p in range(2):
                    ps = psq.tile([128, 512], F32, tag="psq", name=f"q_{qc}_{p}")
                    for i in range(DK):
                        nc.tensor.matmul(
                            ps[:], wqkv_sb[:, i, p * 128:(p + 1) * 128],
                            x_sb[:, i, qc * 512:(qc + 1) * 512],
                            start=(i == 0), stop=(i == DK - 1),
                        )
                    nc.vector.tensor_copy(out=q_sb[p][:, qc * 512:(qc + 1) * 512], in_=ps[:])
                for p in range(2):
                    ps = psq.tile([128, 512], F32, tag="psq", name=f"k_{qc}_{p}")
                    for i in range(DK):
                        nc.tensor.matmul(
                            ps[:], wqkv_sb[:, i, 256 + p * 128:256 + (p + 1) * 128],
                            x_sb[:, i, qc * 512:(qc + 1) * 512],
                            start=(i == 0), stop=(i == DK - 1),
                        )
                    nc.vector.tensor_copy(out=k_sb[p][:, qc * 512:(qc + 1) * 512], in_=ps[:])

            def emit_qkv_v(qc):
                for kb in range(qc * 4, qc * 4 + 4):
                    ps = psq.tile([128, 512], F32, tag="psq", name=f"v_{kb}")
                    for i in range(DK):
                        nc.tensor.matmul(
                            ps[:, 0:256], x_sb[:, i, kb * 128:(kb + 1) * 128],
                            wqkv_sb[:, i, 512:768],
                            start=(i == 0), stop=(i == DK - 1),
                        )
                    nc.vector.tensor_copy(
                        out=v_sb[:, kb, :, 0:64],
                        in_=ps[:, 0:256].rearrange("p (h d) -> p h d", h=4),
                    )

            def emit_attn(qc, p):
                kbs = act[qc]
                y_ps = [psy.tile([65, 512], F32, tag="psy", name=f"yps{p}_{qc}_{hh}") for hh in range(2)]
                for idx, (kb, w0, w1, moff) in enumerate(kbs):
                    w = w1 - w0
                    first, last = idx == 0, idx == len(kbs) - 1
                    s_ps = pss.tile([128, 2, 512], F32, tag="pss", name=f"s_{p}_{qc}_{kb}")
                    for hh in range(2):
                        lo = hh * 64
                        nc.tensor.matmul(
                            s_ps[:, hh, w0:w1],
                            k_sb[p][lo:lo + 64, kb * 128:(kb + 1) * 128],
                            q_sb[p][lo:lo + 64, qc * 512 + w0:qc * 512 + w1],
                            start=True, stop=True,
                        )
                    pt = ppool.tile([128, 2, 512], BF16, tag="pt", name=f"pt{p}_{qc}_{kb}")
                    nc.scalar.activation(pt[:, :, w0:w1], s_ps[:, :, w0:w1], Exp, scale=0.125)
                    if moff >= 0:
                        nc.vector.tensor_tensor(
                            out=pt[:, :, w0:w1],
                            in0=pt[:, :, w0:w1],
                            in1=mask_sb[:, None, moff:moff + w].to_broadcast((128, 2, w)),
                            op=mybir.AluOpType.mult,
                        )
                    for hh in range(2):
                        nc.tensor.matmul(
                            y_ps[hh][:, w0:w1], v_sb[:, kb, p * 2 + hh, :],
                            pt[:, hh, w0:w1],
                            start=first, stop=last, skip_group_check=True,
                        )
                # evacuate y to SBUF fast (releases the PSUM banks); the
                # normalization chain is emitted later (deferred one group)
                # so no queue ever stalls on an unmet dependency.
                y_sb = npool.tile([65, 2, 512], F32, tag="ysb", name=f"ysb{qc}_{p}")
                nc.scalar.copy(out=y_sb[:, 0, :], in_=y_ps[0][:])
                nc.scalar.copy(out=y_sb[:, 1, :], in_=y_ps[1][:])
                lp = npool.tile([128, 8], F32, tag="lp", name=f"lp{qc}_{p}")
                nc.sync.dma_start(lp[:], y_sb[64:65, :, :])
                return y_sb, lp

            def emit_norm(qc, p, y_sb, lp):
                # row 64 of y_sb holds the softmax denominators
                nc.vector.reciprocal(lp[:], lp[:])
                l0 = npool.tile([1, 1024], F32, tag="l0")
                nc.sync.dma_start(l0[:], lp[:])
                lb = npool.tile([64, 1024], F32, tag="lb", name=f"lb{qc}_{p}")
                nc.gpsimd.partition_broadcast(lb[:], l0[:])
                nc.vector.tensor_mul(
                    out=y_qc[qc][0:64, p, :], in0=y_sb[0:64, 0, :],
                    in1=lb[:, 0:512])
                yt = npool.tile([64, 512], BF16, tag="yt")
                nc.gpsimd.tensor_tensor(
                    out=yt[:], in0=y_sb[0:64, 1, :],
                    in1=lb[:, 512:1024], op=mybir.AluOpType.mult)
                nc.gpsimd.dma_start(y_qc[qc][64:128, p, :], yt[:])

            def emit_proj(qc):
                for mt in range(qc * 4, qc * 4 + 4):
                    ot = opool.tile([128, 1024], BF16, tag="ot", name=f"ot{mt}")
                    for n in range(2):
                        ps = psq.tile([128, 512], F32, tag="psq", name=f"pso{mt}_{n}")
                        for c in range(2):
                            nc.tensor.matmul(
                                ps[:], y_qc[qc][:, c, (mt % 4) * 128:(mt % 4) * 128 + 128],
                                wp_sb[:, c, n * 512:(n + 1) * 512],
                                start=(c == 0), stop=(c == 1),
                            )
                        if n == 0:
                            nc.vector.tensor_copy(out=ot[:, 0:512], in_=ps[:])
                        elif mt % 2 == 0:
                            nc.scalar.copy(out=ot[:, 512:1024], in_=ps[:])
                        else:
                            nc.vector.tensor_copy(out=ot[:, 512:1024], in_=ps[:])
                    eng = nc.sync if mt % 2 == 0 else nc.scalar
                    eng.dma_start(out[mt * 128:(mt + 1) * 128, :], ot[:])

            # ---- schedule: norm chains deferred one attention group ----
            emit_qkv_qk(0)
            emit_qkv_v(0)
            n00 = emit_attn(0, 0)
            emit_qkv_qk(1)
            n01 = emit_attn(0, 1)
            emit_norm(0, 0, *n00)
            emit_qkv_v(1)
            n10 = emit_attn(1, 0)
            emit_norm(0, 1, *n01)
            emit_qkv_qk(2)
            n11 = emit_attn(1, 1)
            emit_norm(1, 0, *n10)
            emit_qkv_v(2)
            emit_proj(0)
            n20 = emit_attn(2, 0)
            emit_norm(1, 1, *n11)
            emit_qkv_qk(3)
            n21 = emit_attn(2, 1)
            emit_norm(2, 0, *n20)
            emit_qkv_v(3)
            emit_proj(1)
            n30 = emit_attn(3, 0)
            emit_norm(2, 1, *n21)
            n31 = emit_attn(3, 1)
            emit_norm(3, 0, *n30)
            emit_proj(2)
            emit_norm(3, 1, *n31)
            emit_proj(3)

    nc.compile()
    return nc


def _in_maps(x, seg, Wqkv, Wproj, mask_arrs):
    maps = []
    for c in range(8):
        b, g = divmod(c, 4)
        h0 = g * 4
        cs, ce = h0 * 64, h0 * 64 + 256
        maps.append({
            "xT": np.ascontiguousarray(x[b].T).astype(nbf),
            "wqkv": np.ascontiguousarray(np.concatenate(
                [Wqkv[:, cs:ce], Wqkv[:, D + cs:D + ce], Wqkv[:, 2 * D + cs:2 * D + ce]],
                axis=1)).astype(nbf),
            "wp": np.ascontiguousarray(Wproj[cs:ce, :]).astype(nbf),
            "mask": mask_arrs[b],
        })
    return maps


_CACHE = {}


def _prepare(x, segment_ids, W_qkv, W_proj):
    x = np.asarray(x, np.float32)
    seg = np.asarray(segment_ids)
    Wqkv = np.asarray(W_qkv, np.float32)
    Wproj = np.asarray(W_proj, np.float32)
    tiles, mask_arrs, wtot = _schedule(seg)
    key = (tuple((qc, t) for qc in tiles for t in tiles[qc]), wtot)
    if key not in _CACHE:
        _CACHE[key] = _build(tiles, wtot)
    nc = _CACHE[key]
    return nc, _in_maps(x, seg, Wqkv, Wproj, mask_arrs)


def kernel(x, segment_ids, W_qkv, W_proj):
    nc, in_maps = _prepare(x, segment_ids, W_qkv, W_proj)
    res = bass_utils.run_bass_kernel_spmd(nc, in_maps, core_ids=list(range(8)))
    out = np.zeros((B, T, D), np.float32)
    for c in range(8):
        out[c // 4] += res.results[c]["out"].astype(np.float32)
    return out
